# revision 1
# baseline (speedup 1.0000x reference)
"""Trainium2 Bass kernel for EntityAttentionLayer.

entities[4096,128,256] -> fused QKV (W_in [1536,256]) -> 4-head attention
(queries = first 32 entities, pre_mask True => -inf logits) -> out proj
(W_out [512,512] + b_out) -> post_mask True => 0.

Data-parallel over batch across 8 NeuronCores (512 batch elems per core).
Self-contained: hardcodes all shapes; builds + compiles the Bass program on
first call and caches it at module level.
"""

import math
import sys

import numpy as np

for _p in ("/opt/trn_rl_repo", "/root/.axon_site/_ro/trn_rl_repo"):
    if _p not in sys.path:
        sys.path.insert(0, _p)

import concourse.bass as bass
import concourse.tile as tile
from concourse import bacc, mybir
from concourse.bass_utils import run_bass_kernel_spmd
from concourse.masks import make_identity

F32 = mybir.dt.float32
F32R = mybir.dt.float32r
U8 = mybir.dt.uint8

N_CORES = 8
B = 512           # batch elems per core
NE = 128          # entities
NQ = 32           # queries
IN = 256          # input dim
E = 512           # embed dim
H = 4             # heads
HD = 128          # head dim
OUT = 512         # out dim
SCALE = 1.0 / math.sqrt(HD)
NEG_BIG = -1.0e30

# dtype knob for the projection matmuls (f32r = full-speed PE, reduced mantissa).
# Tensors consumed by f32r matmuls must be *produced* in f32r (BIR verifier),
# so the weight tiles / entT8 / ao4 are allocated in this dtype directly.
PROJ_DT = F32R    # Q/K/V projections + out projection


class _Balancer:
    """Round-robin PSUM->SBUF copies between scalar (ACT) and vector (DVE)
    engines weighted by their modeled busy-ns, so neither becomes the
    bottleneck."""

    def __init__(self, nc):
        self.nc = nc
        self.act = 0.0
        self.dve = 0.0

    def copy(self, out, in_):
        fd = 1
        for s in in_.shape[1:]:
            fd *= s
        act_cost = (172.0 + fd) / 1.2
        dve_cost = (120.0 + fd) / 0.96
        if self.act + act_cost <= self.dve + dve_cost:
            self.act += act_cost
            self.nc.scalar.copy(out=out, in_=in_)
        else:
            self.dve += dve_cost
            self.nc.vector.tensor_copy(out=out, in_=in_)

    def charge_act(self, ns):
        self.act += ns

    def charge_dve(self, ns):
        self.dve += ns


def _build(b_per_core=B):
    nb = b_per_core
    assert nb % 8 == 0
    nc = bacc.Bacc(None, target_bir_lowering=False, debug=False)

    ent_d = nc.dram_tensor("entities", [nb, NE, IN], F32, kind="ExternalInput").ap()
    pre_d = nc.dram_tensor("pre_mask", [nb, NQ, NE], U8, kind="ExternalInput").ap()
    post_d = nc.dram_tensor("post_mask", [nb, NQ], U8, kind="ExternalInput").ap()
    winT_d = nc.dram_tensor("w_inT", [IN, 3 * E], PROJ_DT, kind="ExternalInput").ap()
    woutT_d = nc.dram_tensor("w_outT", [E, OUT], PROJ_DT, kind="ExternalInput").ap()
    bout_d = nc.dram_tensor("b_out", [OUT], F32, kind="ExternalInput").ap()
    out_d = nc.dram_tensor("out", [nb, NQ, OUT], F32, kind="ExternalOutput").ap()

    n_g8 = nb // 8       # groups of 8 batch elems
    n_g4 = nb // 4       # groups of 4 (post-mask column index)

    with tile.TileContext(nc) as tc:
        with (
            tc.tile_pool(name="const", bufs=1) as cpool,
            tc.tile_pool(name="ent", bufs=2) as p_ent,
            tc.tile_pool(name="entT", bufs=2) as p_entT,
            tc.tile_pool(name="qkv", bufs=2) as p_qkv,
            tc.tile_pool(name="attn", bufs=2) as p_attn,
            tc.tile_pool(name="small", bufs=3) as p_small,
            tc.tile_pool(name="outb", bufs=2) as p_out,
            tc.tile_pool(name="psum", bufs=8, space="PSUM") as p_ps,
        ):
            bal = _Balancer(nc)

            # ---- constants ----
            winT_sb = cpool.tile([128, 2, 3 * E], PROJ_DT)
            for kc in range(2):
                nc.sync.dma_start(
                    out=winT_sb[:, kc, :], in_=winT_d[kc * 128:(kc + 1) * 128, :]
                )
            woutT_sb = cpool.tile([128, 4, OUT], PROJ_DT)
            for ec in range(4):
                nc.sync.dma_start(
                    out=woutT_sb[:, ec, :], in_=woutT_d[ec * 128:(ec + 1) * 128, :]
                )
            bias_rep = cpool.tile([128, OUT], F32)
            nc.sync.dma_start(
                out=bias_rep,
                in_=bass.AP(tensor=bout_d.tensor, offset=bout_d.offset,
                            ap=[[0, 128], [1, OUT]]),
            )
            ident = cpool.tile([128, 128], F32)
            make_identity(nc, ident)
            negbig = cpool.tile([128, 4, NE], F32)
            nc.vector.memset(negbig, NEG_BIG)

            # notpostT[:, g] = (1 - post_mask) for 4-batch group g, rows (b%4)*32+q.
            # Natural [n_g4, 128] load has partition p = group g (128 u8 per row);
            # one PE transpose yields [row=(b%4)*32+q, g].
            assert n_g4 <= 128
            npost_u8 = cpool.tile([n_g4, 128], U8)
            nc.sync.dma_start(
                out=npost_u8,
                in_=post_d.rearrange("b q -> (b q)").rearrange(
                    "(p f) -> p f", p=n_g4
                ),
            )
            npost_f = cpool.tile([n_g4, 128], F32)
            nc.scalar.activation(
                out=npost_f, in_=npost_u8,
                func=mybir.ActivationFunctionType.Copy, scale=-1.0, bias=1.0,
            )
            notpostT_sb = cpool.tile([128, n_g4], F32)
            ps_np = p_ps.tile([128, n_g4], F32, tag="ps")
            nc.tensor.transpose(ps_np, npost_f, ident[:n_g4, :n_g4])
            bal.copy(out=notpostT_sb, in_=ps_np)

            # ---- main loop over groups of 8 batch elements ----
            for g8 in range(n_g8):
                b8 = g8 * 8

                ent8 = p_ent.tile([128, 8, IN], F32, tag="ent8")
                nc.sync.dma_start(
                    out=ent8, in_=ent_d[b8:b8 + 8].rearrange("b e i -> e b i")
                )

                # transpose entities: entT8[:, kc, j, :] = ent8[:, j, kc-block].T
                entT8 = p_entT.tile([128, 2, 8, NE], PROJ_DT, tag="entT8")
                for kc in range(2):
                    for jh in range(2):
                        ps_t = p_ps.tile([128, 512], F32, tag="ps")
                        for j4 in range(4):
                            j = jh * 4 + j4
                            nc.tensor.transpose(
                                ps_t[:, j4 * 128:(j4 + 1) * 128],
                                ent8[:, j, kc * 128:(kc + 1) * 128],
                                ident,
                            )
                        bal.copy(
                            out=entT8[:, kc, jh * 4:(jh + 1) * 4, :].rearrange(
                                "p b e -> p (b e)"),
                            in_=ps_t,
                        )

                # Q projection into zero-padded per-head stacks:
                # qp[h][:, j, h*32+q] = Q_h^T[d, q]; all other columns are zero,
                # so the logits matmuls can accumulate 4 full-M=128 matmuls
                # (fp32 matmuls cannot write PSUM at partition offsets 32/96,
                # and matmul cost is N cycles regardless of M -> padding is
                # PE-free).
                qp = []
                for h in range(H):
                    t = p_qkv.tile([128, 8, 128], F32, tag=f"qp{h}")
                    nc.gpsimd.memset(t, 0.0)
                    qp.append(t)
                for mc in range(H):
                    ps_q = p_ps.tile([128, 8, NQ], F32, tag="ps")
                    for kc in range(2):
                        nc.tensor.matmul(
                            ps_q.rearrange("p b q -> p (b q)"),
                            winT_sb[:, kc, mc * 128:(mc + 1) * 128],
                            entT8[:, kc, :, 0:NQ],
                            start=(kc == 0), stop=(kc == 1),
                        )
                    bal.copy(
                        out=qp[mc][:, :, mc * NQ:(mc + 1) * NQ],
                        in_=ps_q,
                    )

                # K projection: k8[:, h, j, :] = K_h^T [d, e]
                k8 = p_qkv.tile([128, H, 8, NE], F32, tag="k8")
                for mc in range(H):
                    for jh in range(2):
                        ps_k = p_ps.tile([128, 512], F32, tag="ps")
                        for kc in range(2):
                            nc.tensor.matmul(
                                ps_k,
                                winT_sb[:, kc, E + mc * 128:E + (mc + 1) * 128],
                                entT8[:, kc, jh * 4:(jh + 1) * 4, :],
                                start=(kc == 0), stop=(kc == 1),
                            )
                        bal.copy(
                            out=k8[:, mc, jh * 4:(jh + 1) * 4, :].rearrange(
                                "p b e -> p (b e)"),
                            in_=ps_k,
                        )

                # V projection, [entity, dim] layout: v8[:, j, :] = V [e, 512]
                v8 = p_qkv.tile([128, 8, E], F32, tag="v8")
                for j in range(8):
                    ps_v = p_ps.tile([128, E], F32, tag="ps")
                    for kc in range(2):
                        nc.tensor.matmul(
                            ps_v,
                            entT8[:, kc, j, :],
                            winT_sb[:, kc, 2 * E:3 * E],
                            start=(kc == 0), stop=(kc == 1),
                        )
                    bal.copy(out=v8[:, j, :], in_=ps_v)

                for g4h in range(2):
                    g = g8 * 2 + g4h
                    # pre-mask for these 4 batch elems, replicated per head:
                    # m4[h*32+q, bl, e] = pre_mask[4g+bl, q, e]
                    m4 = p_small.tile([128, 4, NE], U8, tag="m4")
                    for h in range(H):
                        nc.sync.dma_start(
                            out=m4[h * 32:(h + 1) * 32, :, :],
                            in_=pre_d[4 * g:4 * g + 4].rearrange("b q e -> q b e"),
                        )

                    # logits: ps_l[h*32+q, bl, e], 4 accumulating matmuls per
                    # batch elem (head h contributes rows h*32..h*32+32 via the
                    # zero-padded q8 stack)
                    ps_l = p_ps.tile([128, 4, NE], F32, tag="ps")
                    for bl in range(4):
                        j = g4h * 4 + bl
                        for h in range(H):
                            nc.tensor.matmul(
                                ps_l[:, bl, :],
                                qp[h][:, j, :],
                                k8[:, h, j, :],
                                start=(h == 0), stop=(h == 3),
                            )
                    # mask -> -1e30 where pre_mask
                    nc.vector.copy_predicated(ps_l, m4, negbig)
                    bal.charge_dve((120 + 512) / 0.96)

                    # exp (scaled) + per-row sums; no max-subtraction needed
                    # (logits are O(1); fully-masked rows give sum==0 -> attn 0)
                    p4 = p_attn.tile([128, 4, NE], F32, tag="p4")
                    s4 = p_small.tile([128, 4], F32, tag="s4")
                    for bl in range(4):
                        nc.scalar.activation(
                            out=p4[:, bl, :], in_=ps_l[:, bl, :],
                            func=mybir.ActivationFunctionType.Exp,
                            scale=SCALE, accum_out=s4[:, bl:bl + 1],
                        )
                        bal.charge_act((172 + 128) / 1.2)
                    r4 = p_small.tile([128, 4], F32, tag="r4")
                    nc.vector.tensor_scalar_max(r4, s4, 1.0e-30)
                    nc.vector.reciprocal(r4, r4)
                    bal.charge_dve(130.0)
                    for bl in range(4):
                        nc.vector.tensor_scalar_mul(
                            p4[:, bl, :], p4[:, bl, :], r4[:, bl:bl + 1]
                        )
                        bal.charge_dve((58 + 64) / 0.96)

                    # transpose attention probs: pt4[:, bl, :] = [e, (h,q)]
                    ps_pt = p_ps.tile([128, 512], F32, tag="ps")
                    for bl in range(4):
                        nc.tensor.transpose(
                            ps_pt[:, bl * 128:(bl + 1) * 128], p4[:, bl, :], ident
                        )
                    pt4 = p_attn.tile([128, 4, 128], F32, tag="pt4")
                    bal.copy(out=pt4.rearrange("p b x -> p (b x)"), in_=ps_pt)

                    # attn @ V -> attn_out^T: ao4[:, h(=embed chunk), bl, q]
                    # ((bl,q) adjacent so the out-proj stationary AP flattens
                    # to a single free dim)
                    ps_ao = p_ps.tile([128, H, 4, NQ], F32, tag="ps")
                    for bl in range(4):
                        j = g4h * 4 + bl
                        for h in range(H):
                            nc.tensor.matmul(
                                ps_ao[:, h, bl, :],
                                v8[:, j, h * 128:(h + 1) * 128],
                                pt4[:, bl, h * 32:(h + 1) * 32],
                                start=True, stop=True,
                            )
                    ao4 = p_attn.tile([128, H, 4, NQ], PROJ_DT, tag="ao4")
                    bal.copy(out=ao4.rearrange("p h b q -> p (h b q)"),
                             in_=ps_ao.rearrange("p h b q -> p (h b q)"))

                    # out projection: 4 batch elems merged into M=128 lhsT
                    # (rows (bl,q)), 4 accumulating matmuls over embed chunks
                    ps_o = p_ps.tile([128, OUT], F32, tag="ps")
                    for ec in range(4):
                        nc.tensor.matmul(
                            ps_o,
                            ao4[:, ec, :, :].rearrange("p b q -> p (b q)"),
                            woutT_sb[:, ec, :],
                            start=(ec == 0), stop=(ec == 3),
                        )
                    of = p_out.tile([128, OUT], F32, tag="of")
                    nc.vector.tensor_add(of, ps_o, bias_rep)
                    nc.vector.tensor_scalar_mul(
                        of, of, notpostT_sb[:, g:g + 1]
                    )
                    bal.charge_dve((120 + 512) / 0.96 + (58 + 256) / 0.96)
                    nc.sync.dma_start(
                        out=out_d[4 * g:4 * g + 4].rearrange("b q n -> (b q) n"),
                        in_=of,
                    )

    nc.compile()
    return nc


_CACHE = {}


def _get_nc(nb):
    if nb not in _CACHE:
        _CACHE[nb] = _build(nb)
    return _CACHE[nb]


def _make_in_maps(inputs):
    entities = np.asarray(inputs["entities"], dtype=np.float32)
    pre = np.asarray(inputs["pre_mask"]).astype(np.uint8)
    post = np.asarray(inputs["post_mask"]).astype(np.uint8)
    winT = np.ascontiguousarray(np.asarray(inputs["W_in"], dtype=np.float32).T)
    woutT = np.ascontiguousarray(np.asarray(inputs["W_out"], dtype=np.float32).T)
    bout = np.ascontiguousarray(np.asarray(inputs["b_out"], dtype=np.float32))
    bs = entities.shape[0]
    nb = bs // N_CORES
    in_maps = []
    for c in range(N_CORES):
        sl = slice(c * nb, (c + 1) * nb)
        in_maps.append({
            "entities": np.ascontiguousarray(entities[sl]),
            "pre_mask": np.ascontiguousarray(pre[sl]),
            "post_mask": np.ascontiguousarray(post[sl]),
            "w_inT": winT,
            "w_outT": woutT,
            "b_out": bout,
        })
    return in_maps


def kernel(entities, pre_mask, post_mask, W_in, W_out, b_out):
    in_maps = _make_in_maps({
        "entities": entities, "pre_mask": pre_mask, "post_mask": post_mask,
        "W_in": W_in, "W_out": W_out, "b_out": b_out,
    })
    nb = in_maps[0]["entities"].shape[0]
    nc = _get_nc(nb)
    res = run_bass_kernel_spmd(nc, in_maps, list(range(N_CORES)))
    out = np.concatenate([res.results[c]["out"] for c in range(N_CORES)], axis=0)
    return out

